# revision 43
# baseline (speedup 1.0000x reference)
"""Trainium2 Bass kernel for nn_Densenet_with_skip (gauss blur -> sobel ->
angle-binned 8-direction NMS -> gate).

Reformulation (validated vs reference; device rel l2 ~3.5e-3):
  V  = Bv^T X                    (single vertical banded pass, f16 PSUM->SBUF
                                  transposed with replicate row pads)
  S  = cen * (V^T Bh)            (1 horizontal sweep per row-tile)
  gx'= sqrt(T1) gx = sum_d w_d (V^T shifted d) (sqrt(T1) Mhx)   (3 sweeps)
  gy'= gy/sqrt(T1) via {Mhy, -Mhy}/sqrt(T1)                     (2 sweeps)
  masks:
    m0 = |gx'| >= |gy'|      <=>  T1|gx| >= |gy|     (DVE is_ge)
    m2 = T1^2 |gy'| > |gx'|  <=>  T1|gy| > |gx|      (DVE is_gt on T1^2*ay)
    mneg = gx*gy < 0 (diag-bin swap): gx'*gy' on Pool, is_lt 0 on Pool TS
  sel cascade on S-scale pair maxes: P_main -> CP(mneg,P_anti)
    -> CP(m2,P_vert) -> CP(m0,P_horz); then sel *= s/cen (4x TS),
  out = relu(S - sel) * x  (x >= 0 so relu placement is free).

Engine plan (cost-model driven; wall ~= DVE busy + fill):
  PE: banded matmuls ([N, 132/134] packed band windows, 0.9MB of weights);
      per-row-tile 1-bank psums (pb/pgx/pgy + pv) with bufs=2 so PE runs a
      full row-tile ahead of extraction (8 PSUM banks exactly).
  Act: psum->SBUF extraction (tTv, S, gx, gy) + ax/ay Abs + ay2 scale.
  DVE: compares, pair maxes, 3x CopyPredicated cascade, sel/z/relu tail.
  Pool: prod, mneg, final gate multiply (except last image) + SWDGE
      loads/stores. U/D row-shift maps are SBUF->SBUF HWDGE copies.
  Image 0 shifts extraction work Act->DVE (DVE idles during fill);
  the last image keeps its tail on DVE/sync for a short drain.
x is loaded as f16 (host pre-cast), out is stored as f16 and upcast on
the host; both halve DMA volume vs f32.
"""

import sys

import numpy as np

sys.path.insert(0, "/opt/trn_rl_repo")

import concourse.bacc as bacc
import concourse.mybir as mybir
from concourse import tile
from concourse.bass_utils import run_bass_kernel_spmd

N = 512
B_TOTAL = 32
N_CORES = 8
B_CORE = B_TOTAL // N_CORES  # 4 images per core
NCHUNK = N // 128  # 4
W2 = N + 2  # padded segment width (S and tT_v layouts)

F16 = mybir.dt.float16
F32 = mybir.dt.float32

T1 = float(np.tan(np.pi / 8))  # tan(22.5 deg)
ALU = mybir.AluOpType
AF = mybir.ActivationFunctionType


def _band_ranges(halo):
    out = []
    for r in range(NCHUNK):
        lo = max(0, 128 * r - halo)
        hi = min(N, 128 * r + 128 + halo)
        out.append((lo, hi))
    return out

R5 = _band_ranges(2)
R7 = _band_ranges(3)
WMAX5 = 132
WMAX7 = 134


def _banded_sweeps(nc, psum_ap, sweeps, ranges, off=0, fresh=True, final=True):
    """Accumulate several banded sweeps into psum[:, off:off+N].

    sweeps: list of (lhsT_slices[r], w_tile, wmax) where w_tile holds packed
    band windows, chunk r at cols [r*wmax, r*wmax + width_r). If `fresh`,
    sweep 0 initializes every psum column (with overlap splitting); later
    sweeps accumulate. `final` marks the last sweep's last matmul stop=True."""
    nsw = len(sweeps)
    for si, (lhsT, wt, wmax) in enumerate(sweeps):
        covered = 0
        last_sweep = final and si == nsw - 1
        for r in range(NCHUNK):
            lo, hi = ranges[r]
            base = lo  # window start == lo by construction
            woff = r * wmax - base
            stop = last_sweep and r == NCHUNK - 1
            if fresh and si == 0:
                start = r == 0
                if not start and lo < covered:
                    nc.tensor.matmul(
                        psum_ap[:, off + lo : off + covered], lhsT[r],
                        wt[:, woff + lo : woff + covered],
                        start=False, stop=False,
                    )
                    lo = covered
            else:
                start = False
            nc.tensor.matmul(
                psum_ap[:, off + lo : off + hi], lhsT[r],
                wt[:, woff + lo : woff + hi],
                start=start, stop=stop,
            )
            covered = hi


def build_nc(cen, v):
    s = -v          # S = s*b
    zs = cen / s    # z = zs*S - sel

    nc = bacc.Bacc("TRN2", target_bir_lowering=False, debug=False)

    x_d = nc.dram_tensor("x", [B_CORE * N, N], F16, kind="ExternalInput").ap()
    w_specs = [("bv", WMAX5), ("bh", WMAX5), ("mhx", WMAX7),
               ("mhx2", WMAX7), ("mhy", WMAX7), ("mhyn", WMAX7)]
    w_d = {
        k: nc.dram_tensor("w_" + k, [N, wm], F16, kind="ExternalInput").ap()
        for k, wm in w_specs
    }
    out_d = nc.dram_tensor("out", [B_CORE * N, N], F16, kind="ExternalOutput").ap()

    with tile.TileContext(nc) as tc:
        with (
            tc.tile_pool(name="wpool", bufs=1) as wpool,
            tc.tile_pool(name="xpool", bufs=1) as xpool,
            tc.tile_pool(name="tv", bufs=2) as tvpool,
            tc.tile_pool(name="post", bufs=2) as post,
            tc.tile_pool(name="psumv", bufs=2, space="PSUM") as psumv,
            tc.tile_pool(name="psumh", bufs=2, space="PSUM") as psumh,
        ):
            zrow = wpool.tile([1, W2], F16, tag="zrow")
            nc.vector.memset(zrow[:], 0.0)

            # --- weights + x loads, ordered to shorten pipeline fill:
            # img0's x halves and bv first (vertical pass needs only bv),
            # then the horizontal-pass weights, then the remaining images.
            def load_w(k, wm):
                t = wpool.tile([128, NCHUNK * wm], F16, tag=f"w_{k}",
                               name=f"w_{k}")
                nc.sync.dma_start(
                    out=t[:].rearrange("p (q w) -> p q w", w=wm),
                    in_=w_d[k].rearrange("(q p) w -> p q w", p=128),
                )
                return t

            def load_x(i, t, c0, c1, eng=None):
                (eng or nc.gpsimd).dma_start(
                    out=t[:].rearrange("p (q w) -> p q w", w=N)[:, :, c0:c1],
                    in_=x_d[i * N : (i + 1) * N, c0:c1].rearrange(
                        "(q p) w -> p q w", p=128),
                )

            w_sb = {}
            xh = [xpool.tile([128, NCHUNK * N], F16, tag=f"xh_{i}",
                             name=f"xh_{i}") for i in range(B_CORE)]
            load_x(0, xh[0], 0, 256)
            w_sb["bv"] = load_w("bv", WMAX5)
            load_x(0, xh[0], 256, N)
            for k, wm in w_specs:
                if k != "bv":
                    w_sb[k] = load_w(k, wm)
            for i in range(1, B_CORE):
                load_x(i, xh[i], 0, N, eng=nc.sync)

            for i in range(B_CORE):
                # ---------- vertical pass: tTv = (Bv^T x)^T, replicate pads --
                tTv = tvpool.tile([128, NCHUNK * W2], F16, tag="tTv")
                tTv3 = tTv[:].rearrange("p (q w) -> p q w", w=W2)
                for c in range(NCHUNK):  # one column-chunk per 1-bank psum
                    pv = psumv.tile([128, N], F32, tag="pv")
                    lhsT = [xh[i][:, N * r + 128 * c : N * r + 128 * (c + 1)]
                            for r in range(NCHUNK)]
                    _banded_sweeps(nc, pv, [(lhsT, w_sb["bv"], WMAX5)],
                                   R5, off=0)
                    if i == 0 and c < 2:
                        nc.vector.tensor_copy(
                            tTv3[:, c : c + 1, 1 : N + 1],
                            pv[:].rearrange("p (q w) -> p q w", w=N))
                    else:
                        nc.scalar.activation(
                            tTv3[:, c : c + 1, 1 : N + 1],
                            pv[:].rearrange("p (q w) -> p q w", w=N), AF.Copy)
                # replicate row pads (for the Sv/Dv shifts), one strided op per side
                nc.scalar.activation(tTv3[:, :, 0:1], tTv3[:, :, 1:2],
                                     AF.Copy)
                nc.scalar.activation(tTv3[:, :, N + 1 : N + 2],
                                     tTv3[:, :, N : N + 1], AF.Copy)

                # ---------- horizontal sweeps per row-tile pair ----------
                Sq = post.tile([128, NCHUNK * W2], F16, tag="S")
                Upq = post.tile([128, NCHUNK * W2], F16, tag="Up")
                Dnq = post.tile([128, NCHUNK * W2], F16, tag="Dn")
                Sq3 = Sq[:].rearrange("p (q w) -> p q w", w=W2)
                U3v = Upq[:].rearrange("p (q w) -> p q w", w=W2)
                D3v = Dnq[:].rearrange("p (q w) -> p q w", w=W2)
                zr3 = zrow[:].rearrange("p (q w) -> p q w", w=W2)
                nc.vector.memset(Sq3[:, :, 0:1], 0.0)
                nc.vector.memset(Sq3[:, :, N + 1 : N + 2], 0.0)
                axq = post.tile([128, NCHUNK * N], F16, tag="ax")
                gxq = post.tile([128, NCHUNK * N], F16, tag="gxh")
                gyq = post.tile([128, NCHUNK * N], F16, tag="gyh")
                ayq = post.tile([128, NCHUNK * N], F16, tag="ay")
                prodq = post.tile([128, NCHUNK * N], F16, tag="prod")
                U16 = mybir.dt.uint16

                def qt(tag, dt=F16):
                    t = post.tile([128, NCHUNK * N], dt, tag=tag, name=tag)
                    return t, t[:].rearrange("p (q w) -> p q w", w=N)

                ay2q, _ = qt("ay2")
                mnu, _ = qt("mnu")
                m0u, _ = qt("m0u")
                m2u, _ = qt("m2u")
                selq, sel3 = qt("sel")
                pm3q, pm33 = qt("pm3")
                pm2q, pm23 = qt("pm2")
                pm0q, pm03 = qt("pm0")
                zq, z3 = qt("z")
                oq, o3 = qt("o")
                x3 = xh[i][:].rearrange("p (q w) -> p q w", w=N)
                TTv = nc.vector.tensor_tensor
                TTp = nc.gpsimd.tensor_tensor
                TS = nc.vector.tensor_scalar

                def sl(c, row0, d):
                    b = c * W2 + 1 + row0 + d
                    return tTv[:, b : b + 128]

                def half_masks(h):
                    f0, f1 = h * 2 * N, (h + 1) * 2 * N
                    hs = slice(2 * h, 2 * h + 2)
                    # abs via Act (ax) / sign-bit clear (ay)
                    if i == 0:
                        TS(out=axq[:, f0:f1].bitcast(U16),
                           in0=gxq[:, f0:f1].bitcast(U16),
                           scalar1=0x7FFF, scalar2=None, op0=ALU.bitwise_and)
                        TS(out=ayq[:, f0:f1].bitcast(U16),
                           in0=gyq[:, f0:f1].bitcast(U16),
                           scalar1=0x7FFF, scalar2=None, op0=ALU.bitwise_and)
                    else:
                        nc.scalar.activation(axq[:, f0:f1], gxq[:, f0:f1],
                                             AF.Abs)
                        nc.scalar.activation(ayq[:, f0:f1], gyq[:, f0:f1],
                                             AF.Abs)
                    # masks: m0: ax' >= ay'; m2: T1^2 ay' > ax';
                    # mneg: gx*gy < 0 (only consulted in diagonal bins)
                    TTp(out=prodq[:, f0:f1], in0=gxq[:, f0:f1],
                        in1=gyq[:, f0:f1], op=ALU.mult)
                    nc.gpsimd.tensor_scalar(
                        out=mnu[:, f0:f1], in0=prodq[:, f0:f1],
                        scalar1=0.0, scalar2=None, op0=ALU.is_lt)
                    nc.scalar.activation(ay2q[:, f0:f1], ayq[:, f0:f1],
                                         AF.Copy, scale=float(T1 * T1))
                    TTv(out=m0u[:, f0:f1], in0=axq[:, f0:f1],
                        in1=ayq[:, f0:f1], op=ALU.is_ge)
                    TTv(out=m2u[:, f0:f1], in0=ay2q[:, f0:f1],
                        in1=axq[:, f0:f1], op=ALU.is_gt)
                    TTv(out=pm03[:, hs], in0=Sq3[:, hs, 0:N],
                        in1=Sq3[:, hs, 2 : N + 2], op=ALU.max)

                def half_rest(h, use_pool=True):
                    f0, f1 = h * 2 * N, (h + 1) * 2 * N
                    hs = slice(2 * h, 2 * h + 2)
                    TTv(out=sel3[:, hs], in0=D3v[:, hs, 0:N],
                        in1=U3v[:, hs, 2 : N + 2], op=ALU.max)
                    TTv(out=pm33[:, hs], in0=D3v[:, hs, 2 : N + 2],
                        in1=U3v[:, hs, 0:N], op=ALU.max)
                    TTv(out=pm23[:, hs], in0=D3v[:, hs, 1 : N + 1],
                        in1=U3v[:, hs, 1 : N + 1], op=ALU.max)
                    nc.vector.copy_predicated(selq[:, f0:f1],
                                              mnu[:, f0:f1].bitcast(U16),
                                              pm3q[:, f0:f1])
                    nc.vector.copy_predicated(selq[:, f0:f1],
                                              m2u[:, f0:f1].bitcast(U16),
                                              pm2q[:, f0:f1])
                    nc.vector.copy_predicated(selq[:, f0:f1],
                                              m0u[:, f0:f1].bitcast(U16),
                                              pm0q[:, f0:f1])
                    # S is at cen*b scale, sel at cen*P:
                    # out = relu(S - (s/cen)*sel) * x
                    TS(out=selq[:, f0:f1], in0=selq[:, f0:f1],
                       scalar1=float(s / cen), scalar2=None, op0=ALU.mult)
                    TTv(out=z3[:, hs], in0=Sq3[:, hs, 1 : N + 1],
                        in1=sel3[:, hs], op=ALU.subtract)
                    TS(out=zq[:, f0:f1], in0=zq[:, f0:f1],
                       scalar1=0.0, scalar2=None, op0=ALU.max)
                    o_tt = TTp if use_pool else TTv
                    o_tt(out=o3[:, hs], in0=z3[:, hs],
                         in1=x3[:, hs], op=ALU.mult)
                    st_eng = nc.gpsimd if use_pool else nc.sync
                    st_eng.dma_start(
                        out=out_d[i * N + 256 * h : i * N + 256 * (h + 1),
                                  :].rearrange("(q p) w -> p q w", p=128),
                        in_=oq[:].rearrange("p (q w) -> p q w", w=N)[:, hs, :],
                    )

                for rt in range(2 * 2):  # row-tiles, 1-bank psums, bufs=2
                    pb = psumh.tile([128, N], F32, tag="pb")
                    pgx = psumh.tile([128, N], F32, tag="pgx")
                    pgy = psumh.tile([128, N], F32, tag="pgy")
                    row0 = 128 * rt
                    l_m = [sl(c, row0, -1) for c in range(NCHUNK)]
                    l_0 = [sl(c, row0, 0) for c in range(NCHUNK)]
                    l_p = [sl(c, row0, 1) for c in range(NCHUNK)]
                    _banded_sweeps(nc, pb, [(l_0, w_sb["bh"], WMAX5)],
                                   R5, off=0)
                    _banded_sweeps(nc, pgx, [
                        (l_m, w_sb["mhx"], WMAX7),
                        (l_0, w_sb["mhx2"], WMAX7),
                        (l_p, w_sb["mhx"], WMAX7),
                    ], R7, off=0)
                    _banded_sweeps(nc, pgy, [
                        (l_m, w_sb["mhyn"], WMAX7),
                        (l_p, w_sb["mhy"], WMAX7),
                    ], R7, off=0)

                    f0 = rt * N
                    nc.scalar.activation(
                        Sq3[:, rt : rt + 1, 1 : N + 1],
                        pb[:].rearrange("p (q w) -> p q w", w=N),
                        AF.Copy, scale=cen)
                    # psum holds gx' = sqrt(T1)*gx, gy' = gy/sqrt(T1)
                    if i == 0:
                        nc.vector.tensor_copy(gxq[:, f0 : f0 + N], pgx[:])
                        nc.vector.tensor_copy(gyq[:, f0 : f0 + N], pgy[:])
                    else:
                        nc.scalar.activation(gxq[:, f0 : f0 + N], pgx[:],
                                             AF.Copy)
                        nc.scalar.activation(gyq[:, f0 : f0 + N], pgy[:],
                                             AF.Copy)

                    if rt == 1:
                        half_masks(0)
                        # U/D for half 0, except the U seam (needs S quad 2)
                        nc.sync.dma_start(out=U3v[0:127, 0:2, :],
                                          in_=Sq3[1:128, 0:2, :])
                        nc.sync.dma_start(out=U3v[127:128, 0:1, :],
                                          in_=Sq3[0:1, 1:2, :])
                        nc.sync.dma_start(out=D3v[1:128, 0:2, :],
                                            in_=Sq3[0:127, 0:2, :])
                        nc.sync.dma_start(out=D3v[0:1, 1:2, :],
                                            in_=Sq3[127:128, 0:1, :])
                        nc.sync.dma_start(out=D3v[0:1, 0:1, :], in_=zr3)
                    elif rt == 3:
                        half_masks(1)
                        # U seam for half 0 (needs S quad 2), finish half 0
                        nc.sync.dma_start(out=U3v[127:128, 1:2, :],
                                          in_=Sq3[0:1, 2:3, :])
                        half_rest(0, use_pool=True)
                        # U/D for half 1
                        nc.sync.dma_start(out=U3v[0:127, 2:4, :],
                                          in_=Sq3[1:128, 2:4, :])
                        nc.sync.dma_start(out=U3v[127:128, 2:3, :],
                                          in_=Sq3[0:1, 3:4, :])
                        nc.sync.dma_start(out=U3v[127:128, 3:4, :], in_=zr3)
                        nc.sync.dma_start(out=D3v[1:128, 2:4, :],
                                            in_=Sq3[0:127, 2:4, :])
                        nc.sync.dma_start(out=D3v[0:1, 2:4, :],
                                            in_=Sq3[127:128, 1:3, :])
                        half_rest(1, use_pool=(i < B_CORE - 1))

    nc.compile()
    return nc


# ---------------------------------------------------------------------------
# host side
# ---------------------------------------------------------------------------

def _make_band(weights, offsets, pad):
    M = np.zeros((N, N), dtype=np.float64)
    for w, o in zip(weights, offsets):
        idx = np.arange(N)
        src = idx + o
        if pad == "replicate":
            np.add.at(M, (np.clip(src, 0, N - 1), idx), w)
        else:
            ok = (src >= 0) & (src < N)
            np.add.at(M, (src[ok], idx[ok]), w)
    return M


def _pack_band(M, ranges, wmax):
    P = np.zeros((N, wmax), dtype=np.float16)
    for r in range(NCHUNK):
        lo, hi = ranges[r]
        P[128 * r : 128 * (r + 1), : hi - lo] = M[128 * r : 128 * (r + 1),
                                                  lo:hi].astype(np.float16)
    return np.ascontiguousarray(P)


def _host_weights(gauss_kernel):
    gk = np.asarray(gauss_kernel, dtype=np.float64)[0, 0]
    U, sv, Vt = np.linalg.svd(gk)
    assert sv[1] < 1e-5 * sv[0], "gauss kernel not rank-1 separable"
    wv = U[:, 0] * np.sqrt(sv[0])
    wh = Vt[0] * np.sqrt(sv[0])
    if wv.sum() < 0:
        wv, wh = -wv, -wh
    o5 = [-2, -1, 0, 1, 2]
    o3 = [-1, 0, 1]
    Bv = _make_band(wv, o5, "zero")
    Bh = _make_band(wh, o5, "zero")
    Sh = _make_band([1, 2, 1], o3, "replicate")
    Dh = _make_band([-1, 0, 1], o3, "replicate")
    Mhx = Bh @ Dh
    Mhy = Bh @ Sh
    rt1 = float(np.sqrt(T1))
    return {
        "w_bv": _pack_band(Bv, R5, WMAX5),
        "w_bh": _pack_band(Bh, R5, WMAX5),
        "w_mhx": _pack_band(rt1 * Mhx, R7, WMAX7),
        "w_mhx2": _pack_band(2.0 * rt1 * Mhx, R7, WMAX7),
        "w_mhy": _pack_band(Mhy / rt1, R7, WMAX7),
        "w_mhyn": _pack_band(-Mhy / rt1, R7, WMAX7),
    }


_NC_CACHE = {}
LAST_RESULT = None


def kernel(reconst, gauss_kernel, nms_kernel):
    nk = np.asarray(nms_kernel, dtype=np.float64)
    cen = float(nk[0, 0, 1, 1])
    v = float(nk[0, 0, 1, 2])
    pos = [(1, 2), (2, 2), (2, 1), (2, 0), (1, 0), (0, 0), (0, 1), (0, 2)]
    for d, (r, c) in enumerate(pos):
        k = nk[d, 0].copy()
        assert abs(k[1, 1] - cen) < 1e-6 and abs(k[r, c] - v) < 1e-6
        k[1, 1] = 0.0
        k[r, c] = 0.0
        assert np.abs(k).max() < 1e-7
    assert v < 0

    key = (round(cen, 9), round(v, 9))
    if key not in _NC_CACHE:
        _NC_CACHE[key] = build_nc(cen, v)
    nc = _NC_CACHE[key]

    w = _host_weights(gauss_kernel)
    x = np.asarray(reconst, dtype=np.float32).astype(np.float16).reshape(B_TOTAL, N, N)
    in_maps = []
    for core in range(N_CORES):
        m = {"x": np.ascontiguousarray(
            x[core * B_CORE : (core + 1) * B_CORE].reshape(B_CORE * N, N)
        )}
        m.update(w)
        in_maps.append(m)

    res = run_bass_kernel_spmd(nc, in_maps, core_ids=list(range(N_CORES)))
    global LAST_RESULT
    LAST_RESULT = res
    out = np.concatenate(
        [r["out"].reshape(B_CORE, 1, N, N) for r in res.results], axis=0
    )
    return out.astype(np.float32)

